# revision 4
# baseline (speedup 1.0000x reference)
"""CAWformer forward on 8 TRN2 NeuronCores — data parallel over batch.

Math notes (all exact algebraic rewrites of the reference):
  * irfft(xf_i * conj(xf_j)).mean(-1) == s_i * s_j / DM with s = x.sum(-1),
    so the FFT cross-correlation attention is softmax(outer(s, s)/c) @ x.
  * The 8-shift auto-attention: scores_i = <q@Wk, roll_i(x)> (+const that
    cancels in softmax); out = (sum_i p_i roll_i(x)) @ Wv.T @ Wo.T + const.
  * The depthwise smoothing conv is a (T,T) band matrix S; residual embed
    folds to inp[b].T @ (R.T @ emb_W.T) with R = I - S.

v2 performance structure:
  * All matmul operands are fp16 (PE runs 1 row/cycle at any N in fp16;
    fp32r pays 4x below N=256). PSUM accumulation stays fp32.
  * Weights are double-buffered (wp bufs=2) so layer l+1's DMA streams
    during layer l's compute.
  * The VC-block output x is written twice side by side ([x, x], free size
    2*DM) so every circular roll is one contiguous window: the 8 score
    reductions and 8 value matmuls per batch need no split halves.
  * LN variance via E[z^2]-mu^2: the z-producing scalar_tensor_tensor
    accumulates sum(z) for free and an ACT-engine Square pass accumulates
    sum(z^2); rstd = exp(-0.5*ln(var+eps)) keeps ln+exp+square+identity in
    ONE table set (natural_log_exp_and_others) so the only ACT table
    switches are to/from Gelu, each prewarmed behind FFN matmul phases.
  * Elementwise work is spread across DVE / ACT / Pool to shorten the
    cross-engine critical path.
"""

import os
import numpy as np

B, T, C, DM, L, P, KS = 16, 512, 128, 512, 3, 64, 25
EPS = 1e-5
NS = DM // P           # 8 circular shifts
NC_ = 8                # cores
BPC = B // NC_         # batches per core = 2
H = 2 * DM             # FFN hidden = 1024
KD = DM // 128         # 4 k-tiles over d_model
KH = H // 128          # 8 k-tiles over hidden


def _build(nc, tile, mybir, bass):
    F32 = mybir.dt.float32
    F16 = mybir.dt.float16
    AT = mybir.ActivationFunctionType
    ALU = mybir.AluOpType
    AX = mybir.AxisListType

    def mm(out, lhsT, rhs, start, stop):
        nc.tensor.matmul(out, lhsT, rhs, start=start, stop=stop)

    # ---------------- DRAM I/O ----------------
    d = {}
    def din(name, shape, dt_):
        d[name] = nc.dram_tensor(name, list(shape), dt_, kind="ExternalInput")
        return d[name]

    # weight layouts are pre-shuffled on host to (128, k, n) so every DMA
    # is 128 partitions x contiguous-per-partition (full-bandwidth descriptors)
    din("xin", (128, BPC, KD, C), F16)
    din("memb", (128, KD, DM), F16)
    din("wpos", (C, DM), F32)
    din("ident", (128, 128), F16)
    din("vw1t", (L, 128, KD, H), F16); din("vb1", (L, 128, KH), F32)
    din("vw2t", (L, 128, KH, DM), F16); din("vb2", (L, DM), F16)
    din("aw1t", (L, 128, KD, H), F16); din("ab1", (L, 128, KH), F32)
    din("aw2t", (L, 128, KH, DM), F16); din("ab2", (L, DM), F16)
    din("m1", (L, 128, KD, DM), F16); din("c1", (L, DM), F16)
    din("m2", (L, 128, KD, DM), F16); din("c2", (L, DM), F16)
    din("vsb", (L, 1), F32); din("asb", (L, 1), F32)
    din("vgc", (L, C), F32); din("vbc", (L, C), F32)
    din("vbch", (L, C), F16)
    din("agc", (L, C), F32); din("abc", (L, C), F32)
    din("vgl", (L, DM), F16); din("vbl", (L, DM), F16)
    din("agl", (L, DM), F16); din("abl", (L, DM), F16)
    out_d = nc.dram_tensor("out", [BPC, C, DM], F32, kind="ExternalOutput")

    def bc_ap(src, parts=128):
        # broadcast a DRAM vector AP across partitions
        return bass.AP(tensor=src.tensor, offset=src.offset,
                       ap=[[0, parts]] + [list(x) for x in src.ap])

    def col_ap(src):
        # DRAM vector (n,) -> (n,1) partition-major AP
        return bass.AP(tensor=src.tensor, offset=src.offset,
                       ap=[list(src.ap[0]), [0, 1]])

    inv_sqc = float(1.0 / (DM ** 0.75))

    with tile.TileContext(nc) as tc:
        import contextlib
        ctx = contextlib.ExitStack()
        with ctx:
            wp = ctx.enter_context(tc.tile_pool(name="wp", bufs=2))
            ap_ = ctx.enter_context(tc.tile_pool(name="ap", bufs=1))
            bcp = ctx.enter_context(tc.tile_pool(name="bcp", bufs=16))
            sp = ctx.enter_context(tc.tile_pool(name="sp", bufs=8))
            cp = ctx.enter_context(tc.tile_pool(name="cp", bufs=1))
            pbig = ctx.enter_context(tc.tile_pool(name="pbig", bufs=3, space="PSUM"))
            ph = ctx.enter_context(tc.tile_pool(name="ph", bufs=2, space="PSUM"))
            pt = ctx.enter_context(tc.tile_pool(name="pt", bufs=2, space="PSUM"))

            # ---------------- constants ----------------
            ident = cp.tile([128, 128], F16)
            nc.sync.dma_start(out=ident, in_=d["ident"].ap())
            epsc = cp.tile([128, 1], F32)
            nc.vector.memset(epsc, EPS)
            dum = sp.tile([128, 1], F32, tag="dum", bufs=4)
            # prewarm the ln/exp table set before any real ACT work
            nc.scalar.activation(dum, epsc, AT.Ln)
            dum2 = sp.tile([128, 1], F32, tag="dum", bufs=4)
            nc.scalar.activation(dum2, epsc, AT.Exp)
            memb_sb = cp.tile([128, KD, DM], F16)
            nc.sync.dma_start(out=memb_sb, in_=d["memb"].ap())
            wpos_sb = cp.tile([128, DM], F32)
            nc.sync.dma_start(out=wpos_sb, in_=d["wpos"].ap())
            xin_sb = cp.tile([128, BPC, KD, C], F16)
            nc.sync.dma_start(out=xin_sb, in_=d["xin"].ap())

            # ---------------- embed:  x[c] = xin[c].T @ memb + wpos ----------------
            x_t = ap_.tile([128, BPC, DM], F16, tag="xa", bufs=2)
            for c in range(BPC):
                x_ps = pbig.tile([128, DM], F32, tag="big")
                for k in range(KD):
                    mm(x_ps, xin_sb[:, c, k, :], memb_sb[:, k, :],
                       start=(k == 0), stop=(k == KD - 1))
                nc.vector.tensor_add(x_t[:, c, :], x_ps, wpos_sb)

            phase = os.environ.get("KPHASE", "full")
            srow_of = {}

            # ---------------- layers ----------------
            for l in range(L if phase == "full" else 1):
                if phase == "emb":
                    break
                # ---- layer weight loads (wp bufs=2 -> prefetch overlap) ----
                vw1t = wp.tile([128, KD, H], F16, tag="vw1t")
                nc.sync.dma_start(out=vw1t, in_=d["vw1t"][l])
                vb1 = sp.tile([128, KH], F32, tag="vb1", bufs=2)
                nc.sync.dma_start(out=vb1, in_=d["vb1"][l])
                m1 = wp.tile([128, KD, DM], F16, tag="m1")
                nc.sync.dma_start(out=m1, in_=d["m1"][l])
                m2 = wp.tile([128, KD, DM], F16, tag="m2")
                nc.sync.dma_start(out=m2, in_=d["m2"][l])
                vw2t = wp.tile([128, KH, DM], F16, tag="vw2t")
                nc.sync.dma_start(out=vw2t, in_=d["vw2t"][l])
                aw1t = wp.tile([128, KD, H], F16, tag="aw1t")
                nc.sync.dma_start(out=aw1t, in_=d["aw1t"][l])
                ab1 = sp.tile([128, KH], F32, tag="ab1", bufs=2)
                nc.sync.dma_start(out=ab1, in_=d["ab1"][l])
                aw2t = wp.tile([128, KH, DM], F16, tag="aw2t")
                nc.sync.dma_start(out=aw2t, in_=d["aw2t"][l])

                vgc = sp.tile([128, 1], F32, tag="vgc", bufs=2)
                nc.gpsimd.dma_start(out=vgc, in_=col_ap(d["vgc"][l]))
                agc = sp.tile([128, 1], F32, tag="agc", bufs=2)
                nc.gpsimd.dma_start(out=agc, in_=col_ap(d["agc"][l]))
                vbc = sp.tile([128, 1], F32, tag="vbc", bufs=2)
                nc.gpsimd.dma_start(out=vbc, in_=col_ap(d["vbc"][l]))
                abc = sp.tile([128, 1], F32, tag="abc", bufs=2)
                nc.gpsimd.dma_start(out=abc, in_=col_ap(d["abc"][l]))
                vbcf = bcp.tile([128, 128], F16, tag="bc2", name=f"vbcf{l}")
                nc.gpsimd.dma_start(out=vbcf, in_=bc_ap(d["vbch"][l]))

                def bcast(name):
                    t = bcp.tile([128, DM], F16, tag="bc", name=f"{name}_bc{l}")
                    nc.gpsimd.dma_start(out=t, in_=bc_ap(d[name][l]))
                    return t
                c1b = bcast("c1"); c2b = bcast("c2")
                vb2b = bcast("vb2"); ab2b = bcast("ab2")
                vglb = bcast("vgl"); vblb = bcast("vbl")
                aglb = bcast("agl"); ablb = bcast("abl")

                # gcI = diag(gc_vc) as dense tile for the "+I" residual fold
                gcI = sp.tile([128, 128], F16, tag="gcI", bufs=2)
                nc.vector.tensor_scalar_mul(gcI, ident, vgc)
                vsb = sp.tile([128, 1], F32, tag="vsb", bufs=2)
                nc.gpsimd.dma_start(out=vsb, in_=bc_ap(d["vsb"][l]))
                asb = sp.tile([128, 1], F32, tag="asb", bufs=2)
                nc.gpsimd.dma_start(out=asb, in_=bc_ap(d["asb"][l]))

                # ============ VarCor block ============
                # s = rowsum(x) * 1/DM^0.75 (split sqrt per side)
                cT = ap_.tile([128, BPC, 128], F16, tag="cT", bufs=2)
                for c in range(BPC):
                    if c in srow_of:
                        srow = srow_of[c]
                    else:
                        srow = sp.tile([128, 1], F32, tag="srow", bufs=4)
                        nc.vector.tensor_reduce(srow, x_t[:, c, :], AX.X, ALU.add)
                    s2 = sp.tile([128, 1], F16, tag="s2", bufs=4)
                    nc.vector.tensor_scalar_mul(s2, srow, inv_sqc)
                    sT_ps = pt.tile([1, 128], F16, tag="t", name=f"sTps{l}_{c}")
                    nc.tensor.transpose(sT_ps, s2, ident)
                    sT = sp.tile([1, 128], F16, tag="sT", bufs=4)
                    nc.scalar.activation(sT, sT_ps, AT.Identity)
                    corr_ps = pbig.tile([128, 128], F32, tag="big", name=f"corrps{l}_{c}")
                    mm(corr_ps, sT, sT, start=True, stop=True)
                    # softmax over free axis (values are O(1): skip max-sub)
                    # + BN row-scale + +I fold
                    corrE = ap_.tile([128, 128], F32, tag="corrE", bufs=2)
                    rsum = sp.tile([128, 1], F32, tag="rsum", bufs=4)
                    nc.scalar.activation(corrE, corr_ps, AT.Exp, accum_out=rsum)
                    rinv = sp.tile([128, 1], F32, tag="rinv", bufs=4)
                    nc.vector.reciprocal(rinv, rsum)
                    corrBN = ap_.tile([128, 128], F16, tag="corrBN", bufs=2)
                    nc.vector.tensor_scalar(corrBN, corrE, rinv, vgc, ALU.mult, ALU.mult)
                    nc.vector.tensor_add(corrBN, corrBN, gcI)
                    cT_ps = pt.tile([128, 128], F16, tag="t", name=f"cTps{l}_{c}")
                    nc.tensor.transpose(cT_ps, corrBN, ident)
                    nc.vector.tensor_copy(cT[:, c, :], cT_ps)

                # r2 rows-major and feature-major via two matmul sets
                r2r = ap_.tile([128, BPC, DM], F16, tag="r2r", bufs=2)
                r2T = ap_.tile([128, KD, 2 * 128], F16, tag="r2T", bufs=2)
                for c in range(BPC):
                    rr_ps = pbig.tile([128, DM], F32, tag="big", name=f"rrps{l}_{c}")
                    mm(rr_ps, cT[:, c, :], x_t[:, c, :DM], start=True, stop=True)
                    nc.scalar.activation(r2r[:, c, :], rr_ps, AT.Identity, bias=vbc)
                    for m in range(KD):
                        rt_ps = pt.tile([128, 128], F32, tag="t", name=f"rtps{l}_{c}_{m}")
                        mm(rt_ps, x_t[:, c, m * 128:(m + 1) * 128],
                           cT[:, c, :], start=True, stop=True)
                        # feature-major r2T: BN beta is along the free (channel)
                        # axis here, so add it via a partition-broadcast tile
                        # (GPSIMD cannot read PSUM, so these stay on DVE)
                        nc.vector.tensor_add(r2T[:, m, c * 128:(c + 1) * 128],
                                             rt_ps, vbcf)

                if phase == "corr":
                    x_t = r2r
                    break
                x_t = _ffn_ln(nc, tile, mybir, bass, tc, ap_, sp, bcp, ph, pbig,
                              r2T, r2r, vw1t, vb1, vw2t, vb2b, vglb, vblb, l, "v", epsc,
                              vsb, srow_of, dup=True, last=False)
                if phase == "vc0":
                    break

                # ============ Auto-attention block ============
                # x_t is [128, BPC, 2*DM] ([x, x] duplicated): window sh:sh+DM
                # is roll_sh(x). xT feature-major from the first copy.
                xT = ap_.tile([128, KD, 2 * 128], F16, tag="xT", bufs=2)
                for c in range(BPC):
                    for m in range(KD):
                        tp = pt.tile([128, 128], F16, tag="t", name=f"xTps{l}_{c}_{m}")
                        nc.tensor.transpose(tp, x_t[:, c, m * 128:(m + 1) * 128], ident)
                        if (c * KD + m) % 2 == 0:
                            nc.vector.tensor_copy(xT[:, m, c * 128:(c + 1) * 128], tp)
                        else:
                            nc.scalar.activation(xT[:, m, c * 128:(c + 1) * 128], tp,
                                                 AT.Identity)

                # u = x @ M1 + c1   (rows-major out)
                u_t = ap_.tile([128, BPC, DM], F16, tag="u", bufs=2)
                for c in range(BPC):
                    u_ps = pbig.tile([128, DM], F32, tag="big", name=f"ups{l}_{c}")
                    for k in range(KD):
                        mm(u_ps, xT[:, k, c * 128:(c + 1) * 128],
                           m1[:, k, :], start=(k == 0), stop=(k == KD - 1))
                    nc.vector.tensor_add(u_t[:, c, :], u_ps, c1b)

                if phase == "u":
                    x_t = u_t
                    break

                # scores S[r,i] = <u, roll_i(x)> * DM^-0.5 ; softmax over i
                scl = float(DM ** -0.5)
                Sp_t = ap_.tile([128, BPC, NS], F32, tag="Sp", bufs=2)
                for c in range(BPC):
                    # NOTE: tensor_tensor_reduce wedges the device on this
                    # walrus/NRT build (NRT_EXEC_UNIT_UNRECOVERABLE); use
                    # scalar_tensor_tensor's accum_out instead.
                    Sa = sp.tile([128, NS], F32, tag="Sa", bufs=2)
                    for i in range(NS):
                        trash = ap_.tile([128, DM], F16, tag="trash", bufs=2,
                                         name=f"tr{l}_{c}_{i}")
                        nc.vector.scalar_tensor_tensor(
                            out=trash, in0=u_t[:, c, :], scalar=scl,
                            in1=x_t[:, c, P * i:P * i + DM],
                            op0=ALU.mult, op1=ALU.mult, accum_out=Sa[:, i:i + 1])
                    Se = sp.tile([128, NS], F32, tag="Se", bufs=2)
                    ssum = sp.tile([128, 1], F32, tag="ssum", bufs=4)
                    nc.scalar.activation(Se, Sa, AT.Exp, accum_out=ssum)
                    sinv = sp.tile([128, 1], F32, tag="sinv", bufs=4)
                    nc.vector.reciprocal(sinv, ssum)
                    nc.vector.tensor_scalar(Sp_t[:, c, :], Se, sinv, None, ALU.mult)

                if phase == "sc":
                    xs = ap_.tile([128, BPC, DM], F32, tag="scdump", bufs=1)
                    nc.vector.memset(xs, 0.0)
                    for c in range(BPC):
                        nc.vector.tensor_copy(xs[:, c, 0:NS], Sp_t[:, c, :])
                    x_t = xs
                    break

                # vm = sum_i p_i roll_i(x) via diag matmuls accumulating in PSUM
                vm_t = ap_.tile([128, BPC, DM], F16, tag="vm", bufs=2)
                for c in range(BPC):
                    vm_ps = pbig.tile([128, DM], F32, tag="big", name=f"vmps{l}_{c}")
                    for i in range(NS):
                        dg = ap_.tile([128, 128], F16, tag="dg", bufs=3,
                                      name=f"dg{l}_{c}_{i}")
                        nc.gpsimd.tensor_scalar_mul(dg, ident, Sp_t[:, c, i:i + 1])
                        mm(vm_ps, dg, x_t[:, c, P * i:P * i + DM],
                           start=(i == 0), stop=(i == NS - 1))
                    nc.vector.tensor_copy(vm_t[:, c, :], vm_ps)

                if phase == "vm":
                    x_t = vm_t
                    break

                # vmT feature-major
                vmT = ap_.tile([128, KD, 2 * 128], F16, tag="vmT", bufs=2)
                for c in range(BPC):
                    for m in range(KD):
                        tp2 = pt.tile([128, 128], F16, tag="t", name=f"vmTps{l}_{c}_{m}")
                        nc.tensor.transpose(tp2, vm_t[:, c, m * 128:(m + 1) * 128], ident)
                        if (c * KD + m) % 2 == 0:
                            nc.vector.tensor_copy(vmT[:, m, c * 128:(c + 1) * 128], tp2)
                        else:
                            nc.scalar.activation(vmT[:, m, c * 128:(c + 1) * 128], tp2,
                                                 AT.Identity)

                # attn out rows-major: o = vm @ M2 + c2 ; r1 = BN(o + x)
                r1r = ap_.tile([128, BPC, DM], F16, tag="r1r", bufs=2)
                for c in range(BPC):
                    o_ps = pbig.tile([128, DM], F32, tag="big", name=f"ops{l}_{c}")
                    for k in range(KD):
                        mm(o_ps, vmT[:, k, c * 128:(c + 1) * 128],
                           m2[:, k, :], start=(k == 0), stop=(k == KD - 1))
                    t1 = ap_.tile([128, DM], F32, tag="t1", bufs=2, name=f"t1{l}_{c}")
                    nc.vector.tensor_add(t1, o_ps, x_t[:, c, :DM])
                    nc.vector.tensor_add(t1, t1, c2b)
                    nc.scalar.activation(r1r[:, c, :], t1, AT.Identity, bias=abc, scale=agc)

                if phase == "attn":
                    x_t = r1r
                    break

                # r1T feature-major
                r1T = ap_.tile([128, KD, 2 * 128], F16, tag="r1T", bufs=2)
                for c in range(BPC):
                    for m in range(KD):
                        tp3 = pt.tile([128, 128], F16, tag="t", name=f"r1Tps{l}_{c}_{m}")
                        nc.tensor.transpose(tp3, r1r[:, c, m * 128:(m + 1) * 128], ident)
                        if (c * KD + m) % 2 == 0:
                            nc.vector.tensor_copy(r1T[:, m, c * 128:(c + 1) * 128], tp3)
                        else:
                            nc.scalar.activation(r1T[:, m, c * 128:(c + 1) * 128], tp3,
                                                 AT.Identity)

                x_t = _ffn_ln(nc, tile, mybir, bass, tc, ap_, sp, bcp, ph, pbig,
                              r1T, r1r, aw1t, ab1, aw2t, ab2b, aglb, ablb, l, "a", epsc,
                              asb, srow_of, dup=False, last=(l == L - 1))

            # ---------------- store ----------------
            if x_t.dtype != F32:
                xf = ap_.tile([128, BPC, DM], F32, tag="xf32", bufs=1)
                for c in range(BPC):
                    nc.vector.tensor_copy(xf[:, c, :], x_t[:, c, :DM])
                x_t = xf
            for c in range(BPC):
                nc.sync.dma_start(out=out_d.ap()[c], in_=x_t[:, c, :DM])


def _ffn_ln(nc, tile, mybir, bass, tc, ap_, sp, bcp, ph, pbig,
            rT, rrows, w1t, b1, w2t, b2b, glb, blb, l, pfx, epsc,
            sumb, srow_of, dup, last):
    """h = gelu(r @ W1.T + b1); y = h @ W2.T + b2; x = LN(y + r) * g + b.

    LN stats: the z-producing stt accumulates sum(z); an ACT Square pass
    accumulates sum(z^2); var = E[z^2] - mu^2; rstd = exp(-0.5*ln(var+eps))
    (ln/exp/square/identity live in one ACT table set).

    dup=True: write x twice side by side ([x, x], free 2*DM) so circular
    rolls of the following attention block are contiguous windows.

    Also emits (for the "a" blocks feeding the next varcor) the row-sum of
    the next x via <xn, g> + sum(b) so the correlation chain never waits on
    the gamma/beta affine."""
    F32 = mybir.dt.float32
    F16 = mybir.dt.float16
    AT = mybir.ActivationFunctionType
    ALU = mybir.AluOpType

    # rb = r + b2 precomputed off the critical path (Pool) while FFN runs
    rb = ap_.tile([128, BPC, DM], F16, tag=f"rb{pfx}", bufs=2, name=f"rb{pfx}{l}")
    for c in range(BPC):
        nc.gpsimd.tensor_add(rb[:, c, :], rrows[:, c, :], b2b)

    # prewarm the Gelu table set while the first FFN1 matmuls run
    dg_ = sp.tile([128, 1], F32, tag="dum", bufs=4, name=f"dumg{pfx}{l}")
    nc.scalar.activation(dg_, epsc, AT.Gelu)

    hT = ap_.tile([128, KH, 2 * 128], F16, tag="hT", bufs=2, name=f"hT{pfx}{l}")
    for mh2 in range(KH // 2):
        h_ps = ph.tile([128, 2, 128 * 2], F32, tag="h", name=f"hps{pfx}{l}_{mh2}")
        for half in range(2):
            mh = mh2 * 2 + half
            for k in range(KD):
                nc.tensor.matmul(h_ps[:, half, :], w1t[:, k, mh * 128:(mh + 1) * 128],
                                 rT[:, k, :], start=(k == 0), stop=(k == KD - 1))
            nc.scalar.activation(hT[:, mh, :], h_ps[:, half, :], AT.Gelu,
                                 bias=b1[:, mh:mh + 1])
    # prewarm the ln/exp set back in while FFN2 matmuls run
    dn_ = sp.tile([128, 1], F32, tag="dum", bufs=4, name=f"dumn{pfx}{l}")
    nc.scalar.activation(dn_, epsc, AT.Ln)

    out_w = 2 * DM if dup else DM
    out_dt = F32 if last else F16
    x_new = ap_.tile([128, BPC, out_w], out_dt, tag=f"x{pfx}{'d' if dup else ''}",
                     bufs=2, name=f"x{pfx}{l}")
    for c in range(BPC):
        y_ps = pbig.tile([128, DM], F32, tag="big", name=f"yps{pfx}{l}_{c}")
        for k in range(KH):
            nc.tensor.matmul(y_ps, hT[:, k, c * 128:(c + 1) * 128],
                             w2t[:, k, :], start=(k == 0), stop=(k == KH - 1))
        # z = y + r + b2 (one stt, accumulating sum(z) for the LN mean)
        z = ap_.tile([128, DM], F16, tag="z", bufs=4, name=f"z{pfx}{l}_{c}")
        zsum = sp.tile([128, 1], F32, tag="zsum", bufs=4)
        nc.vector.scalar_tensor_tensor(
            out=z, in0=y_ps, scalar=1.0, in1=rb[:, c, :],
            op0=ALU.mult, op1=ALU.add, accum_out=zsum)
        # sum(z^2) on the ACT engine (square is in every table set)
        ztr = ap_.tile([128, DM], F16, tag="ztr", bufs=2, name=f"ztr{pfx}{l}_{c}")
        z2sum = sp.tile([128, 1], F32, tag="z2sum", bufs=4)
        nc.scalar.activation(ztr, z, AT.Square, accum_out=z2sum)
        mu = sp.tile([128, 1], F32, tag="mu", bufs=4)
        nc.vector.tensor_scalar_mul(mu, zsum, float(1.0 / DM))
        nb = sp.tile([128, 1], F32, tag="nb", bufs=4)
        nc.vector.tensor_scalar_mul(nb, mu, -1.0)
        mu2 = sp.tile([128, 1], F32, tag="mu2", bufs=4)
        nc.vector.tensor_mul(mu2, mu, mu)
        var = sp.tile([128, 1], F32, tag="var", bufs=4)
        nc.vector.scalar_tensor_tensor(
            out=var, in0=z2sum, scalar=float(1.0 / DM), in1=mu2,
            op0=ALU.mult, op1=ALU.subtract)
        # rstd = exp(-0.5 * ln(var + eps))  (stays in the ln/exp table set)
        lv = sp.tile([128, 1], F32, tag="lv", bufs=4)
        nc.scalar.activation(lv, var, AT.Ln, bias=epsc)
        rstd = sp.tile([128, 1], F32, tag="rstd", bufs=4)
        nc.scalar.activation(rstd, lv, AT.Exp, scale=-0.5)
        xn = ap_.tile([128, DM], F16, tag="xn", bufs=2, name=f"xn{pfx}{l}_{c}")
        nc.vector.tensor_scalar(xn, z, nb, rstd, ALU.add, ALU.mult)
        if pfx == "a" and l < L - 1:
            # next-layer corr row-sum: <xn, g> + sum(b) — skips the affine
            trash2 = ap_.tile([128, DM], F16, tag="tr2", bufs=2,
                              name=f"tr2{pfx}{l}_{c}")
            sraw = sp.tile([128, 1], F32, tag="sraw", bufs=4)
            nc.vector.scalar_tensor_tensor(
                out=trash2, in0=xn, scalar=1.0, in1=glb,
                op0=ALU.mult, op1=ALU.mult, accum_out=sraw)
            srow = sp.tile([128, 1], F32, tag="srow", bufs=4, name=f"srow{pfx}{l}_{c}")
            nc.scalar.activation(srow, sraw, AT.Identity, bias=sumb)
            srow_of[c] = srow
        # affine (the next-layer corr chain does not wait on it: srow above)
        nc.vector.tensor_mul(x_new[:, c, :DM], xn, glb)
        nc.vector.tensor_add(x_new[:, c, :DM], x_new[:, c, :DM], blb)
        if dup:
            # second copy for contiguous roll windows (off critical path)
            nc.gpsimd.tensor_copy(x_new[:, c, DM:], x_new[:, c, :DM])
    return x_new


# ======================================================================
# host side
# ======================================================================

_COMPILED = {}


def _compile():
    if "nc" in _COMPILED:
        return _COMPILED["nc"]
    import concourse.bass as bass
    import concourse.bacc as bacc
    import concourse.tile as tile
    from concourse import mybir
    nc = bacc.Bacc("TRN2", target_bir_lowering=False, debug=False, num_devices=NC_)
    _build(nc, tile, mybir, bass)
    nc.compile()
    _COMPILED["nc"] = nc
    return nc


def _host_prep(inputs):
    f = lambda k: np.asarray(inputs[k], np.float32)
    ld_w = f("ld_w").reshape(KS).astype(np.float64)
    # conv matrix with replicate padding, R = I - S
    S = np.zeros((T, T), np.float64)
    idx = np.clip(np.arange(T)[:, None] + np.arange(KS)[None, :] - KS // 2, 0, T - 1)
    for k in range(KS):
        np.add.at(S, (np.arange(T), idx[:, k]), ld_w[k])
    Rm = np.eye(T) - S
    emb_W = f("emb_W").astype(np.float64)
    memb = (Rm.T @ emb_W.T).astype(np.float16)              # (T, DM)
    wpos = (f("W_pos") + f("emb_b")[None, :]
            - float(f("ld_b")[0]) * emb_W.sum(1).astype(np.float32)[None, :])

    g = {"memb": np.ascontiguousarray(memb.reshape(KD, 128, DM).transpose(1, 0, 2)),
         "wpos": np.ascontiguousarray(wpos.astype(np.float32)),
         "ident": np.eye(128, dtype=np.float16)}

    s1 = np.float32(1.0 / np.sqrt(1.0 + EPS))
    def stack(fn, dt=np.float32):
        return np.ascontiguousarray(np.stack([fn(l) for l in range(L)]).astype(dt))

    def shuf(a):
        # (k*128, n) -> (128, k, n): SBUF layout with contiguous per-partition rows
        kn, n = a.shape
        return a.reshape(kn // 128, 128, n).transpose(1, 0, 2)

    h16 = np.float16
    g["vw1t"] = stack(lambda l: shuf(f("vc_W1")[l].T), h16)
    g["vb1"] = stack(lambda l: f("vc_b1")[l].reshape(KH, 128).T)
    g["vw2t"] = stack(lambda l: shuf(f("vc_W2")[l].T), h16)
    g["vb2"] = stack(lambda l: f("vc_b2")[l], h16)
    g["aw1t"] = stack(lambda l: shuf(f("aa_W1")[l].T), h16)
    g["ab1"] = stack(lambda l: f("aa_b1")[l].reshape(KH, 128).T)
    g["aw2t"] = stack(lambda l: shuf(f("aa_W2")[l].T), h16)
    g["ab2"] = stack(lambda l: f("aa_b2")[l], h16)
    g["m1"] = stack(lambda l: shuf(f("aa_Wq")[l].astype(np.float64).T @ f("aa_Wk")[l].astype(np.float64)), h16)
    g["c1"] = stack(lambda l: f("aa_bq")[l].astype(np.float64) @ f("aa_Wk")[l].astype(np.float64), h16)
    g["m2"] = stack(lambda l: shuf((f("aa_Wo")[l].astype(np.float64) @ f("aa_Wv")[l].astype(np.float64)).T), h16)
    g["c2"] = stack(lambda l: f("aa_bv")[l].astype(np.float64) @ f("aa_Wo")[l].astype(np.float64).T
                    + f("aa_bo")[l].astype(np.float64), h16)
    g["vsb"] = stack(lambda l: f("vc_ln_b")[l].sum(keepdims=True))
    g["asb"] = stack(lambda l: f("aa_ln_b")[l].sum(keepdims=True))
    g["vgc"] = stack(lambda l: f("vc_bn_g")[l] * s1)
    g["vbc"] = stack(lambda l: f("vc_bn_b")[l])
    g["vbch"] = stack(lambda l: f("vc_bn_b")[l], h16)
    g["vgl"] = stack(lambda l: f("vc_ln_g")[l], h16)
    g["vbl"] = stack(lambda l: f("vc_ln_b")[l], h16)
    g["agc"] = stack(lambda l: f("aa_bn_g")[l] * s1)
    g["abc"] = stack(lambda l: f("aa_bn_b")[l])
    g["agl"] = stack(lambda l: f("aa_ln_g")[l], h16)
    g["abl"] = stack(lambda l: f("aa_ln_b")[l], h16)
    return g


def kernel(**inputs):
    from concourse.bass_utils import run_bass_kernel_spmd
    nc = _compile()
    g = _host_prep(inputs)
    inp = np.asarray(inputs["inp"], np.float32)
    in_maps = []
    for core in range(NC_):
        m = dict(g)
        sl = inp[core * BPC:(core + 1) * BPC]          # (BPC, T, C)
        m["xin"] = np.ascontiguousarray(
            sl.reshape(BPC, KD, 128, C).transpose(2, 0, 1, 3)).astype(np.float16)
        in_maps.append(m)
    res = run_bass_kernel_spmd(nc, in_maps, core_ids=list(range(NC_)))
    if res.exec_time_ns is not None:
        kernel.last_exec_time_ns = res.exec_time_ns
    out = np.concatenate([res.results[k]["out"] for k in range(NC_)], axis=0)
    return out


kernel.last_exec_time_ns = None


# revision 10
# speedup vs baseline: 1.4822x; 1.4822x over previous
"""CAWformer forward on 8 TRN2 NeuronCores — data parallel over batch.

Math notes (all exact algebraic rewrites of the reference):
  * irfft(xf_i * conj(xf_j)).mean(-1) == s_i * s_j / DM with s = x.sum(-1),
    so the FFT cross-correlation attention is softmax(outer(s, s)/c) @ x.
  * The 8-shift auto-attention: scores_i = <q@Wk, roll_i(x)> (+const that
    cancels in softmax); out = (sum_i p_i roll_i(x)) @ Wv.T @ Wo.T + const.
  * The depthwise smoothing conv is a (T,T) band matrix S; residual embed
    folds to inp[b].T @ (R.T @ emb_W.T) with R = I - S.

v2 performance structure:
  * All matmul operands are fp16 (PE runs 1 row/cycle at any N in fp16;
    fp32r pays 4x below N=256). PSUM accumulation stays fp32.
  * Weights are double-buffered (wp bufs=2) so layer l+1's DMA streams
    during layer l's compute.
  * The VC-block output x is written twice side by side ([x, x], free size
    2*DM) so every circular roll is one contiguous window: the 8 score
    reductions and 8 value matmuls per batch need no split halves.
  * LN variance via E[z^2]-mu^2: the z-producing scalar_tensor_tensor
    accumulates sum(z) for free and an ACT-engine Square pass accumulates
    sum(z^2); rstd = exp(-0.5*ln(var+eps)) keeps ln+exp+square+identity in
    ONE table set (natural_log_exp_and_others) so the only ACT table
    switches are to/from Gelu, each prewarmed behind FFN matmul phases.
  * Elementwise work is spread across DVE / ACT / Pool to shorten the
    cross-engine critical path.
"""

import os
import numpy as np

B, T, C, DM, L, P, KS = 16, 512, 128, 512, 3, 64, 25
EPS = 1e-5
NS = DM // P           # 8 circular shifts
NC_ = 8                # cores
BPC = B // NC_         # batches per core = 2
H = 2 * DM             # FFN hidden = 1024
KD = DM // 128         # 4 k-tiles over d_model
KH = H // 128          # 8 k-tiles over hidden


def _build(nc, tile, mybir, bass):
    F32 = mybir.dt.float32
    F16 = mybir.dt.float16
    AT = mybir.ActivationFunctionType
    ALU = mybir.AluOpType
    AX = mybir.AxisListType

    def mm(out, lhsT, rhs, start, stop):
        nc.tensor.matmul(out, lhsT, rhs, start=start, stop=stop)

    # ---------------- DRAM I/O ----------------
    d = {}
    def din(name, shape, dt_):
        d[name] = nc.dram_tensor(name, list(shape), dt_, kind="ExternalInput")
        return d[name]

    # weight layouts are pre-shuffled on host to (128, k, n) so every DMA
    # is 128 partitions x contiguous-per-partition (full-bandwidth descriptors)
    din("xin", (128, BPC, KD, C), F16)
    din("memb", (128, KD, DM), F16)
    din("wpos", (C, DM), F32)
    din("ident", (128, 128), F16)
    din("vw1t", (L, 128, KD, H), F16); din("vb1", (L, 128, KH), F32)
    din("vw2t", (L, 128, KH, DM), F16); din("vb2", (L, DM), F16)
    din("aw1t", (L, 128, KD, H), F16); din("ab1", (L, 128, KH), F32)
    din("aw2t", (L, 128, KH, DM), F16); din("ab2", (L, DM), F16)
    din("m1", (L, 128, KD, DM), F16); din("c1", (L, DM), F16)
    din("m2", (L, 128, KD, DM), F16); din("c2", (L, DM), F16)
    din("vsb", (L, 1), F32); din("asb", (L, 1), F32)
    din("vgc", (L, C), F32); din("vbc", (L, C), F32)
    din("vbch", (L, C), F16)
    din("agc", (L, C), F32); din("abc", (L, C), F32)
    din("vgl", (L, DM), F16); din("vbl", (L, DM), F16)
    din("agl", (L, DM), F16); din("abl", (L, DM), F16)
    out_d = nc.dram_tensor("out", [BPC, C, DM], F32, kind="ExternalOutput")

    def bc_ap(src, parts=128):
        # broadcast a DRAM vector AP across partitions
        return bass.AP(tensor=src.tensor, offset=src.offset,
                       ap=[[0, parts]] + [list(x) for x in src.ap])

    def col_ap(src):
        # DRAM vector (n,) -> (n,1) partition-major AP
        return bass.AP(tensor=src.tensor, offset=src.offset,
                       ap=[list(src.ap[0]), [0, 1]])

    inv_sqc = float(1.0 / (DM ** 0.75))

    with tile.TileContext(nc) as tc:
        import contextlib
        ctx = contextlib.ExitStack()
        with ctx:
            wp = ctx.enter_context(tc.tile_pool(name="wp", bufs=2))
            ap_ = ctx.enter_context(tc.tile_pool(name="ap", bufs=1))
            bcp = ctx.enter_context(tc.tile_pool(name="bcp", bufs=16))
            sp = ctx.enter_context(tc.tile_pool(name="sp", bufs=8))
            cp = ctx.enter_context(tc.tile_pool(name="cp", bufs=1))
            pbig = ctx.enter_context(tc.tile_pool(name="pbig", bufs=3, space="PSUM"))
            ph = ctx.enter_context(tc.tile_pool(name="ph", bufs=2, space="PSUM"))
            pt = ctx.enter_context(tc.tile_pool(name="pt", bufs=2, space="PSUM"))

            # ---------------- constants ----------------
            ident = cp.tile([128, 128], F16)
            nc.sync.dma_start(out=ident, in_=d["ident"].ap())
            epsc = cp.tile([128, 1], F32)
            nc.vector.memset(epsc, EPS)
            dum = sp.tile([128, 1], F32, tag="dum", bufs=4)
            # prewarm the exp table set before the first corr softmax
            nc.scalar.activation(dum, epsc, AT.Exp)
            memb_sb = cp.tile([128, KD, DM], F16)
            nc.sync.dma_start(out=memb_sb, in_=d["memb"].ap())
            wpos_sb = cp.tile([128, DM], F32)
            nc.sync.dma_start(out=wpos_sb, in_=d["wpos"].ap())
            xin_sb = cp.tile([128, BPC, KD, C], F16)
            nc.sync.dma_start(out=xin_sb, in_=d["xin"].ap())

            # ---------------- embed:  x[c] = xin[c].T @ memb + wpos ----------------
            x_t = ap_.tile([128, BPC, DM], F16, tag="xa", bufs=2)
            for c in range(BPC):
                x_ps = pbig.tile([128, DM], F32, tag="big")
                for k in range(KD):
                    mm(x_ps, xin_sb[:, c, k, :], memb_sb[:, k, :],
                       start=(k == 0), stop=(k == KD - 1))
                nc.vector.tensor_add(x_t[:, c, :], x_ps, wpos_sb)

            phase = os.environ.get("KPHASE", "full")
            srow_of = {}

            # ---------------- layers ----------------
            for l in range(L if phase == "full" else 1):
                if phase == "emb":
                    break
                # ---- layer weight loads (wp bufs=2 -> prefetch overlap) ----
                vw1t = wp.tile([128, KD, H], F16, tag="vw1t")
                nc.sync.dma_start(out=vw1t, in_=d["vw1t"][l])
                vb1 = sp.tile([128, KH], F32, tag="vb1", bufs=2)
                nc.sync.dma_start(out=vb1, in_=d["vb1"][l])
                m1 = wp.tile([128, KD, DM], F16, tag="m1")
                nc.sync.dma_start(out=m1, in_=d["m1"][l])
                m2 = wp.tile([128, KD, DM], F16, tag="m2")
                nc.sync.dma_start(out=m2, in_=d["m2"][l])
                vw2t = wp.tile([128, KH, DM], F16, tag="vw2t")
                nc.sync.dma_start(out=vw2t, in_=d["vw2t"][l])
                aw1t = wp.tile([128, KD, H], F16, tag="aw1t")
                nc.sync.dma_start(out=aw1t, in_=d["aw1t"][l])
                ab1 = sp.tile([128, KH], F32, tag="ab1", bufs=2)
                nc.sync.dma_start(out=ab1, in_=d["ab1"][l])
                aw2t = wp.tile([128, KH, DM], F16, tag="aw2t")
                nc.sync.dma_start(out=aw2t, in_=d["aw2t"][l])

                vgc = sp.tile([128, 1], F32, tag="vgc", bufs=2)
                nc.gpsimd.dma_start(out=vgc, in_=col_ap(d["vgc"][l]))
                agc = sp.tile([128, 1], F32, tag="agc", bufs=2)
                nc.gpsimd.dma_start(out=agc, in_=col_ap(d["agc"][l]))
                vbc = sp.tile([128, 1], F32, tag="vbc", bufs=2)
                nc.gpsimd.dma_start(out=vbc, in_=col_ap(d["vbc"][l]))
                abc = sp.tile([128, 1], F32, tag="abc", bufs=2)
                nc.gpsimd.dma_start(out=abc, in_=col_ap(d["abc"][l]))
                vbcf = bcp.tile([128, 128], F16, tag="bc2", name=f"vbcf{l}")
                nc.gpsimd.dma_start(out=vbcf, in_=bc_ap(d["vbch"][l]))

                def bcast(name):
                    t = bcp.tile([128, DM], F16, tag="bc", name=f"{name}_bc{l}")
                    nc.gpsimd.dma_start(out=t, in_=bc_ap(d[name][l]))
                    return t
                c1b = bcast("c1"); c2b = bcast("c2")
                vb2b = bcast("vb2"); ab2b = bcast("ab2")
                vglb = bcast("vgl"); vblb = bcast("vbl")
                aglb = bcast("agl"); ablb = bcast("abl")

                # gcI = diag(gc_vc) as dense tile for the "+I" residual fold
                gcI = sp.tile([128, 128], F16, tag="gcI", bufs=2)
                nc.vector.tensor_scalar_mul(gcI, ident, vgc)
                vsb = sp.tile([128, 1], F32, tag="vsb", bufs=2)
                nc.gpsimd.dma_start(out=vsb, in_=bc_ap(d["vsb"][l]))
                asb = sp.tile([128, 1], F32, tag="asb", bufs=2)
                nc.gpsimd.dma_start(out=asb, in_=bc_ap(d["asb"][l]))

                # ============ VarCor block ============
                # s = rowsum(x) * 1/DM^0.75 (split sqrt per side)
                cT = ap_.tile([128, BPC, 128], F16, tag="cT", bufs=2)
                for c in range(BPC):
                    if c in srow_of:
                        srow = srow_of[c]
                    else:
                        srow = sp.tile([128, 1], F32, tag="srow", bufs=4)
                        nc.vector.tensor_reduce(srow, x_t[:, c, :], AX.X, ALU.add)
                    s2 = sp.tile([128, 1], F16, tag="s2", bufs=4)
                    nc.vector.tensor_scalar_mul(s2, srow, inv_sqc)
                    sT_ps = pt.tile([1, 128], F16, tag="t", name=f"sTps{l}_{c}")
                    nc.tensor.transpose(sT_ps, s2, ident)
                    sT = sp.tile([1, 128], F16, tag="sT", bufs=4)
                    nc.scalar.activation(sT, sT_ps, AT.Identity)
                    corr_ps = pbig.tile([128, 128], F32, tag="big", name=f"corrps{l}_{c}")
                    mm(corr_ps, sT, sT, start=True, stop=True)
                    # softmax over free axis (values are O(1): skip max-sub)
                    # + BN row-scale + +I fold
                    corrE = ap_.tile([128, 128], F32, tag="corrE", bufs=2)
                    rsum = sp.tile([128, 1], F32, tag="rsum", bufs=4)
                    nc.scalar.activation(corrE, corr_ps, AT.Exp, accum_out=rsum)
                    rinv = sp.tile([128, 1], F32, tag="rinv", bufs=4)
                    nc.vector.reciprocal(rinv, rsum)
                    corrBN = ap_.tile([128, 128], F16, tag="corrBN", bufs=2)
                    nc.vector.tensor_scalar(corrBN, corrE, rinv, vgc, ALU.mult, ALU.mult)
                    nc.vector.tensor_add(corrBN, corrBN, gcI)
                    cT_ps = pt.tile([128, 128], F16, tag="t", name=f"cTps{l}_{c}")
                    nc.tensor.transpose(cT_ps, corrBN, ident)
                    nc.vector.tensor_copy(cT[:, c, :], cT_ps)

                # r2 rows-major and feature-major via two matmul sets
                r2r = ap_.tile([128, BPC, DM], F16, tag="r2r", bufs=2)
                r2T = ap_.tile([128, KD, 2 * 128], F16, tag="r2T", bufs=2)
                for c in range(BPC):
                    rr_ps = pbig.tile([128, DM], F32, tag="big", name=f"rrps{l}_{c}")
                    mm(rr_ps, cT[:, c, :], x_t[:, c, :DM], start=True, stop=True)
                    nc.scalar.activation(r2r[:, c, :], rr_ps, AT.Identity, bias=vbc)
                    for m in range(KD):
                        rt_ps = pt.tile([128, 128], F32, tag="t", name=f"rtps{l}_{c}_{m}")
                        mm(rt_ps, x_t[:, c, m * 128:(m + 1) * 128],
                           cT[:, c, :], start=True, stop=True)
                        # feature-major r2T: BN beta is along the free (channel)
                        # axis here, so add it via a partition-broadcast tile
                        # (GPSIMD cannot read PSUM, so these stay on DVE)
                        nc.vector.tensor_add(r2T[:, m, c * 128:(c + 1) * 128],
                                             rt_ps, vbcf)

                if phase == "corr":
                    x_t = r2r
                    break
                x_t = _ffn_ln(nc, tile, mybir, bass, tc, ap_, sp, bcp, ph, pbig,
                              r2T, r2r, vw1t, vb1, vw2t, vb2b, vglb, vblb, l, "v", epsc,
                              vsb, srow_of, dup=True, last=False)
                if phase == "vc0":
                    break

                # ============ Auto-attention block ============
                # x_t is [128, BPC, 2*DM] ([x, x] duplicated): window sh:sh+DM
                # is roll_sh(x). xT feature-major from the first copy.
                xT = ap_.tile([128, KD, 2 * 128], F16, tag="xT", bufs=2)
                for c in range(BPC):
                    for m in range(KD):
                        tp = pt.tile([128, 128], F16, tag="t", name=f"xTps{l}_{c}_{m}")
                        nc.tensor.transpose(tp, x_t[:, c, m * 128:(m + 1) * 128], ident)
                        if (c * KD + m) % 2 == 0:
                            nc.vector.tensor_copy(xT[:, m, c * 128:(c + 1) * 128], tp)
                        else:
                            nc.scalar.activation(xT[:, m, c * 128:(c + 1) * 128], tp,
                                                 AT.Identity)

                # u = x @ M1 + c1   (rows-major out)
                u_t = ap_.tile([128, BPC, DM], F16, tag="u", bufs=2)
                for c in range(BPC):
                    u_ps = pbig.tile([128, DM], F32, tag="big", name=f"ups{l}_{c}")
                    for k in range(KD):
                        mm(u_ps, xT[:, k, c * 128:(c + 1) * 128],
                           m1[:, k, :], start=(k == 0), stop=(k == KD - 1))
                    nc.vector.tensor_add(u_t[:, c, :], u_ps, c1b)

                if phase == "u":
                    x_t = u_t
                    break

                # scores S[r,i] = <u, roll_i(x)> * DM^-0.5 ; softmax over i
                scl = float(DM ** -0.5)
                Sp_t = ap_.tile([128, BPC, NS], F32, tag="Sp", bufs=2)
                for c in range(BPC):
                    # NOTE: tensor_tensor_reduce wedges the device on this
                    # walrus/NRT build (NRT_EXEC_UNIT_UNRECOVERABLE); use
                    # scalar_tensor_tensor's accum_out instead.
                    Sa = sp.tile([128, NS], F32, tag="Sa", bufs=2)
                    for i in range(NS):
                        trash = ap_.tile([128, DM], F16, tag="trash", bufs=2,
                                         name=f"tr{l}_{c}_{i}")
                        nc.vector.scalar_tensor_tensor(
                            out=trash, in0=u_t[:, c, :], scalar=scl,
                            in1=x_t[:, c, P * i:P * i + DM],
                            op0=ALU.mult, op1=ALU.mult, accum_out=Sa[:, i:i + 1])
                    Se = sp.tile([128, NS], F32, tag="Se", bufs=2)
                    ssum = sp.tile([128, 1], F32, tag="ssum", bufs=4)
                    nc.scalar.activation(Se, Sa, AT.Exp, accum_out=ssum)
                    sinv = sp.tile([128, 1], F32, tag="sinv", bufs=4)
                    nc.vector.reciprocal(sinv, ssum)
                    nc.vector.tensor_scalar(Sp_t[:, c, :], Se, sinv, None, ALU.mult)

                if phase == "sc":
                    xs = ap_.tile([128, BPC, DM], F32, tag="scdump", bufs=1)
                    nc.vector.memset(xs, 0.0)
                    for c in range(BPC):
                        nc.vector.tensor_copy(xs[:, c, 0:NS], Sp_t[:, c, :])
                    x_t = xs
                    break

                # vm = sum_i p_i roll_i(x) via diag matmuls accumulating in PSUM
                vm_t = ap_.tile([128, BPC, DM], F16, tag="vm", bufs=2)
                for c in range(BPC):
                    vm_ps = pbig.tile([128, DM], F32, tag="big", name=f"vmps{l}_{c}")
                    for i in range(NS):
                        dg = ap_.tile([128, 128], F16, tag="dg", bufs=3,
                                      name=f"dg{l}_{c}_{i}")
                        nc.vector.tensor_scalar_mul(dg, ident, Sp_t[:, c, i:i + 1])
                        mm(vm_ps, dg, x_t[:, c, P * i:P * i + DM],
                           start=(i == 0), stop=(i == NS - 1))
                    nc.vector.tensor_copy(vm_t[:, c, :], vm_ps)

                if phase == "vm":
                    x_t = vm_t
                    break

                # vmT feature-major
                vmT = ap_.tile([128, KD, 2 * 128], F16, tag="vmT", bufs=2)
                for c in range(BPC):
                    for m in range(KD):
                        tp2 = pt.tile([128, 128], F16, tag="t", name=f"vmTps{l}_{c}_{m}")
                        nc.tensor.transpose(tp2, vm_t[:, c, m * 128:(m + 1) * 128], ident)
                        if (c * KD + m) % 2 == 0:
                            nc.vector.tensor_copy(vmT[:, m, c * 128:(c + 1) * 128], tp2)
                        else:
                            nc.scalar.activation(vmT[:, m, c * 128:(c + 1) * 128], tp2,
                                                 AT.Identity)

                # attn out rows-major: o = vm @ M2 + c2 ; r1 = BN(o + x)
                r1r = ap_.tile([128, BPC, DM], F16, tag="r1r", bufs=2)
                for c in range(BPC):
                    o_ps = pbig.tile([128, DM], F32, tag="big", name=f"ops{l}_{c}")
                    for k in range(KD):
                        mm(o_ps, vmT[:, k, c * 128:(c + 1) * 128],
                           m2[:, k, :], start=(k == 0), stop=(k == KD - 1))
                    t1 = ap_.tile([128, DM], F32, tag="t1", bufs=2, name=f"t1{l}_{c}")
                    nc.vector.tensor_add(t1, o_ps, x_t[:, c, :DM])
                    nc.vector.tensor_add(t1, t1, c2b)
                    nc.scalar.activation(r1r[:, c, :], t1, AT.Identity, bias=abc, scale=agc)

                if phase == "attn":
                    x_t = r1r
                    break

                # r1T feature-major
                r1T = ap_.tile([128, KD, 2 * 128], F16, tag="r1T", bufs=2)
                for c in range(BPC):
                    for m in range(KD):
                        tp3 = pt.tile([128, 128], F16, tag="t", name=f"r1Tps{l}_{c}_{m}")
                        nc.tensor.transpose(tp3, r1r[:, c, m * 128:(m + 1) * 128], ident)
                        if (c * KD + m) % 2 == 0:
                            nc.vector.tensor_copy(r1T[:, m, c * 128:(c + 1) * 128], tp3)
                        else:
                            nc.scalar.activation(r1T[:, m, c * 128:(c + 1) * 128], tp3,
                                                 AT.Identity)

                x_t = _ffn_ln(nc, tile, mybir, bass, tc, ap_, sp, bcp, ph, pbig,
                              r1T, r1r, aw1t, ab1, aw2t, ab2b, aglb, ablb, l, "a", epsc,
                              asb, srow_of, dup=False, last=(l == L - 1))

            # ---------------- store ----------------
            if x_t.dtype != F32:
                xf = ap_.tile([128, BPC, DM], F32, tag="xf32", bufs=1)
                for c in range(BPC):
                    nc.vector.tensor_copy(xf[:, c, :], x_t[:, c, :DM])
                x_t = xf
            for c in range(BPC):
                nc.sync.dma_start(out=out_d.ap()[c], in_=x_t[:, c, :DM])


def _ffn_ln(nc, tile, mybir, bass, tc, ap_, sp, bcp, ph, pbig,
            rT, rrows, w1t, b1, w2t, b2b, glb, blb, l, pfx, epsc,
            sumb, srow_of, dup, last):
    """h = gelu(r @ W1.T + b1); y = h @ W2.T + b2; x = LN(y + r) * g + b.

    LN stats: the z-producing stt accumulates sum(z); an ACT Square pass
    accumulates sum(z^2); var = E[z^2] - mu^2; rstd = exp(-0.5*ln(var+eps))
    (ln/exp/square/identity live in one ACT table set).

    dup=True: write x twice side by side ([x, x], free 2*DM) so circular
    rolls of the following attention block are contiguous windows.

    Also emits (for the "a" blocks feeding the next varcor) the row-sum of
    the next x via <xn, g> + sum(b) so the correlation chain never waits on
    the gamma/beta affine."""
    F32 = mybir.dt.float32
    F16 = mybir.dt.float16
    AT = mybir.ActivationFunctionType
    ALU = mybir.AluOpType

    # rb = r + b2 precomputed off the critical path while FFN runs
    rb = ap_.tile([128, BPC, DM], F16, tag=f"rb{pfx}", bufs=2, name=f"rb{pfx}{l}")
    for c in range(BPC):
        nc.vector.tensor_add(rb[:, c, :], rrows[:, c, :], b2b)

    # prewarm the Gelu table set while the first FFN1 matmuls run
    dg_ = sp.tile([128, 1], F32, tag="dum", bufs=4, name=f"dumg{pfx}{l}")
    nc.scalar.activation(dg_, epsc, AT.Gelu)

    hT = ap_.tile([128, KH, 2 * 128], F16, tag="hT", bufs=2, name=f"hT{pfx}{l}")
    for mh2 in range(KH // 2):
        h_ps = ph.tile([128, 2, 128 * 2], F32, tag="h", name=f"hps{pfx}{l}_{mh2}")
        for half in range(2):
            mh = mh2 * 2 + half
            for k in range(KD):
                nc.tensor.matmul(h_ps[:, half, :], w1t[:, k, mh * 128:(mh + 1) * 128],
                                 rT[:, k, :], start=(k == 0), stop=(k == KD - 1))
            nc.scalar.activation(hT[:, mh, :], h_ps[:, half, :], AT.Gelu,
                                 bias=b1[:, mh:mh + 1])
    # prewarm the sqrt set back in while FFN2 matmuls run (square is in
    # every table set, so the Square stats pass below never switches)
    dn_ = sp.tile([128, 1], F32, tag="dum", bufs=4, name=f"dumn{pfx}{l}")
    nc.scalar.activation(dn_, epsc, AT.Sqrt)

    out_w = 2 * DM if dup else DM
    out_dt = F32 if last else F16
    x_new = ap_.tile([128, BPC, out_w], out_dt, tag=f"x{pfx}{'d' if dup else ''}",
                     bufs=2, name=f"x{pfx}{l}")
    for c in range(BPC):
        y_ps = pbig.tile([128, DM], F32, tag="big", name=f"yps{pfx}{l}_{c}")
        for k in range(KH):
            nc.tensor.matmul(y_ps, hT[:, k, c * 128:(c + 1) * 128],
                             w2t[:, k, :], start=(k == 0), stop=(k == KH - 1))
        # z = y + r + b2 (one stt, accumulating sum(z) for the LN mean)
        z = ap_.tile([128, DM], F16, tag="z", bufs=4, name=f"z{pfx}{l}_{c}")
        zsum = sp.tile([128, 1], F32, tag="zsum", bufs=4)
        nc.vector.scalar_tensor_tensor(
            out=z, in0=y_ps, scalar=1.0, in1=rb[:, c, :],
            op0=ALU.mult, op1=ALU.add, accum_out=zsum)
        # sum(z^2) on the ACT engine (square is in every table set)
        ztr = ap_.tile([128, DM], F16, tag="ztr", bufs=2, name=f"ztr{pfx}{l}_{c}")
        z2sum = sp.tile([128, 1], F32, tag="z2sum", bufs=4)
        nc.scalar.activation(ztr, z, AT.Square, accum_out=z2sum)
        mu = sp.tile([128, 1], F32, tag="mu", bufs=4)
        nc.vector.tensor_scalar_mul(mu, zsum, float(1.0 / DM))
        nb = sp.tile([128, 1], F32, tag="nb", bufs=4)
        nc.vector.tensor_scalar_mul(nb, mu, -1.0)
        mu2 = sp.tile([128, 1], F32, tag="mu2", bufs=4)
        nc.vector.tensor_mul(mu2, mu, mu)
        var = sp.tile([128, 1], F32, tag="var", bufs=4)
        nc.vector.scalar_tensor_tensor(
            out=var, in0=z2sum, scalar=float(1.0 / DM), in1=mu2,
            op0=ALU.mult, op1=ALU.subtract)
        # rstd = 1/sqrt(var + eps)  (sqrt prewarmed above; recip is native DVE)
        std = sp.tile([128, 1], F32, tag="std", bufs=4)
        nc.scalar.activation(std, var, AT.Sqrt, bias=epsc)
        rstd = sp.tile([128, 1], F32, tag="rstd", bufs=4)
        nc.vector.reciprocal(rstd, std)
        xn = ap_.tile([128, DM], F16, tag="xn", bufs=2, name=f"xn{pfx}{l}_{c}")
        nc.vector.tensor_scalar(xn, z, nb, rstd, ALU.add, ALU.mult)
        if pfx == "a" and l < L - 1:
            # next-layer corr row-sum: <xn, g> + sum(b) — skips the affine
            trash2 = ap_.tile([128, DM], F16, tag="tr2", bufs=2,
                              name=f"tr2{pfx}{l}_{c}")
            sraw = sp.tile([128, 1], F32, tag="sraw", bufs=4)
            nc.vector.scalar_tensor_tensor(
                out=trash2, in0=xn, scalar=1.0, in1=glb,
                op0=ALU.mult, op1=ALU.mult, accum_out=sraw)
            srow = sp.tile([128, 1], F32, tag="srow", bufs=4, name=f"srow{pfx}{l}_{c}")
            nc.scalar.activation(srow, sraw, AT.Identity, bias=sumb)
            srow_of[c] = srow
        # affine (the next-layer corr chain does not wait on it: srow above)
        nc.vector.tensor_mul(x_new[:, c, :DM], xn, glb)
        nc.vector.tensor_add(x_new[:, c, :DM], x_new[:, c, :DM], blb)
        if dup:
            # second copy for contiguous roll windows (off critical path)
            nc.vector.tensor_copy(x_new[:, c, DM:], x_new[:, c, :DM])
    # prewarm the exp set for the following softmax (scores / next corr)
    de_ = sp.tile([128, 1], F32, tag="dum", bufs=4, name=f"dume{pfx}{l}")
    nc.scalar.activation(de_, epsc, AT.Exp)
    return x_new


# ======================================================================
# host side
# ======================================================================

_COMPILED = {}


def _compile():
    if "nc" in _COMPILED:
        return _COMPILED["nc"]
    import concourse.bass as bass
    import concourse.bacc as bacc
    import concourse.tile as tile
    from concourse import mybir
    nc = bacc.Bacc("TRN2", target_bir_lowering=False, debug=False, num_devices=NC_)
    _build(nc, tile, mybir, bass)
    nc.compile()
    _COMPILED["nc"] = nc
    return nc


def _host_prep(inputs):
    f = lambda k: np.asarray(inputs[k], np.float32)
    ld_w = f("ld_w").reshape(KS).astype(np.float64)
    # conv matrix with replicate padding, R = I - S
    S = np.zeros((T, T), np.float64)
    idx = np.clip(np.arange(T)[:, None] + np.arange(KS)[None, :] - KS // 2, 0, T - 1)
    for k in range(KS):
        np.add.at(S, (np.arange(T), idx[:, k]), ld_w[k])
    Rm = np.eye(T) - S
    emb_W = f("emb_W").astype(np.float64)
    memb = (Rm.T @ emb_W.T).astype(np.float16)              # (T, DM)
    wpos = (f("W_pos") + f("emb_b")[None, :]
            - float(f("ld_b")[0]) * emb_W.sum(1).astype(np.float32)[None, :])

    g = {"memb": np.ascontiguousarray(memb.reshape(KD, 128, DM).transpose(1, 0, 2)),
         "wpos": np.ascontiguousarray(wpos.astype(np.float32)),
         "ident": np.eye(128, dtype=np.float16)}

    s1 = np.float32(1.0 / np.sqrt(1.0 + EPS))
    def stack(fn, dt=np.float32):
        return np.ascontiguousarray(np.stack([fn(l) for l in range(L)]).astype(dt))

    def shuf(a):
        # (k*128, n) -> (128, k, n): SBUF layout with contiguous per-partition rows
        kn, n = a.shape
        return a.reshape(kn // 128, 128, n).transpose(1, 0, 2)

    h16 = np.float16
    g["vw1t"] = stack(lambda l: shuf(f("vc_W1")[l].T), h16)
    g["vb1"] = stack(lambda l: f("vc_b1")[l].reshape(KH, 128).T)
    g["vw2t"] = stack(lambda l: shuf(f("vc_W2")[l].T), h16)
    g["vb2"] = stack(lambda l: f("vc_b2")[l], h16)
    g["aw1t"] = stack(lambda l: shuf(f("aa_W1")[l].T), h16)
    g["ab1"] = stack(lambda l: f("aa_b1")[l].reshape(KH, 128).T)
    g["aw2t"] = stack(lambda l: shuf(f("aa_W2")[l].T), h16)
    g["ab2"] = stack(lambda l: f("aa_b2")[l], h16)
    g["m1"] = stack(lambda l: shuf(f("aa_Wq")[l].astype(np.float64).T @ f("aa_Wk")[l].astype(np.float64)), h16)
    g["c1"] = stack(lambda l: f("aa_bq")[l].astype(np.float64) @ f("aa_Wk")[l].astype(np.float64), h16)
    g["m2"] = stack(lambda l: shuf((f("aa_Wo")[l].astype(np.float64) @ f("aa_Wv")[l].astype(np.float64)).T), h16)
    g["c2"] = stack(lambda l: f("aa_bv")[l].astype(np.float64) @ f("aa_Wo")[l].astype(np.float64).T
                    + f("aa_bo")[l].astype(np.float64), h16)
    g["vsb"] = stack(lambda l: f("vc_ln_b")[l].sum(keepdims=True))
    g["asb"] = stack(lambda l: f("aa_ln_b")[l].sum(keepdims=True))
    g["vgc"] = stack(lambda l: f("vc_bn_g")[l] * s1)
    g["vbc"] = stack(lambda l: f("vc_bn_b")[l])
    g["vbch"] = stack(lambda l: f("vc_bn_b")[l], h16)
    g["vgl"] = stack(lambda l: f("vc_ln_g")[l], h16)
    g["vbl"] = stack(lambda l: f("vc_ln_b")[l], h16)
    g["agc"] = stack(lambda l: f("aa_bn_g")[l] * s1)
    g["abc"] = stack(lambda l: f("aa_bn_b")[l])
    g["agl"] = stack(lambda l: f("aa_ln_g")[l], h16)
    g["abl"] = stack(lambda l: f("aa_ln_b")[l], h16)
    return g


def kernel(**inputs):
    from concourse.bass_utils import run_bass_kernel_spmd
    nc = _compile()
    g = _host_prep(inputs)
    inp = np.asarray(inputs["inp"], np.float32)
    in_maps = []
    for core in range(NC_):
        m = dict(g)
        sl = inp[core * BPC:(core + 1) * BPC]          # (BPC, T, C)
        m["xin"] = np.ascontiguousarray(
            sl.reshape(BPC, KD, 128, C).transpose(2, 0, 1, 3)).astype(np.float16)
        in_maps.append(m)
    res = run_bass_kernel_spmd(nc, in_maps, core_ids=list(range(NC_)))
    if res.exec_time_ns is not None:
        kernel.last_exec_time_ns = res.exec_time_ns
    out = np.concatenate([res.results[k]["out"] for k in range(NC_)], axis=0)
    return out


kernel.last_exec_time_ns = None


# revision 13
# speedup vs baseline: 1.5125x; 1.0205x over previous
"""CAWformer forward on 8 TRN2 NeuronCores — data parallel over batch.

Math notes (all exact algebraic rewrites of the reference):
  * irfft(xf_i * conj(xf_j)).mean(-1) == s_i * s_j / DM with s = x.sum(-1),
    so the FFT cross-correlation attention is softmax(outer(s, s)/c) @ x.
  * The 8-shift auto-attention: scores_i = <q@Wk, roll_i(x)> (+const that
    cancels in softmax); out = (sum_i p_i roll_i(x)) @ Wv.T @ Wo.T + const.
  * The depthwise smoothing conv is a (T,T) band matrix S; residual embed
    folds to inp[b].T @ (R.T @ emb_W.T) with R = I - S.

v2 performance structure:
  * All matmul operands are fp16 (PE runs 1 row/cycle at any N in fp16;
    fp32r pays 4x below N=256). PSUM accumulation stays fp32.
  * Weights are double-buffered (wp bufs=2) so layer l+1's DMA streams
    during layer l's compute.
  * The VC-block output x is written twice side by side ([x, x], free size
    2*DM) so every circular roll is one contiguous window: the 8 score
    reductions and 8 value matmuls per batch need no split halves.
  * LN variance via E[z^2]-mu^2: the z-producing scalar_tensor_tensor
    accumulates sum(z) for free and an ACT-engine Square pass accumulates
    sum(z^2); rstd = exp(-0.5*ln(var+eps)) keeps ln+exp+square+identity in
    ONE table set (natural_log_exp_and_others) so the only ACT table
    switches are to/from Gelu, each prewarmed behind FFN matmul phases.
  * Elementwise work is spread across DVE / ACT / Pool to shorten the
    cross-engine critical path.
"""

import os
import numpy as np

B, T, C, DM, L, P, KS = 16, 512, 128, 512, 3, 64, 25
EPS = 1e-5
NS = DM // P           # 8 circular shifts
NC_ = 8                # cores
BPC = B // NC_         # batches per core = 2
H = 2 * DM             # FFN hidden = 1024
KD = DM // 128         # 4 k-tiles over d_model
KH = H // 128          # 8 k-tiles over hidden


def _build(nc, tile, mybir, bass):
    F32 = mybir.dt.float32
    F16 = mybir.dt.float16
    AT = mybir.ActivationFunctionType
    ALU = mybir.AluOpType
    AX = mybir.AxisListType

    def mm(out, lhsT, rhs, start, stop):
        nc.tensor.matmul(out, lhsT, rhs, start=start, stop=stop)

    # ---------------- DRAM I/O ----------------
    d = {}
    def din(name, shape, dt_):
        d[name] = nc.dram_tensor(name, list(shape), dt_, kind="ExternalInput")
        return d[name]

    # weight layouts are pre-shuffled on host to (128, k, n) so every DMA
    # is 128 partitions x contiguous-per-partition (full-bandwidth descriptors)
    din("xin", (128, BPC, KD, C), F16)
    din("memb", (128, KD, DM), F16)
    din("wpos", (C, DM), F32)
    din("ident", (128, 128), F16)
    din("vw1t", (L, 128, KD, H), F16); din("vb1", (L, 128, KH), F32)
    din("vw2t", (L, 128, KH, DM), F16); din("vb2", (L, DM), F16)
    din("aw1t", (L, 128, KD, H), F16); din("ab1", (L, 128, KH), F32)
    din("aw2t", (L, 128, KH, DM), F16); din("ab2", (L, DM), F16)
    din("m1", (L, 128, KD, DM), F16); din("c1", (L, DM), F16)
    din("m2", (L, 128, KD, DM), F16); din("c2", (L, DM), F16)
    din("vsb", (L, 1), F32); din("asb", (L, 1), F32)
    din("vgc", (L, C), F32); din("vbc", (L, C), F32)
    din("vbch", (L, C), F16)
    din("agc", (L, C), F32); din("abc", (L, C), F32)
    din("vgl", (L, DM), F16); din("vbl", (L, DM), F16)
    din("agl", (L, DM), F16); din("abl", (L, DM), F16)
    out_d = nc.dram_tensor("out", [BPC, C, DM], F32, kind="ExternalOutput")

    def bc_ap(src, parts=128):
        # broadcast a DRAM vector AP across partitions
        return bass.AP(tensor=src.tensor, offset=src.offset,
                       ap=[[0, parts]] + [list(x) for x in src.ap])

    def col_ap(src):
        # DRAM vector (n,) -> (n,1) partition-major AP
        return bass.AP(tensor=src.tensor, offset=src.offset,
                       ap=[list(src.ap[0]), [0, 1]])

    inv_sqc = float(1.0 / (DM ** 0.75))

    with tile.TileContext(nc) as tc:
        import contextlib
        ctx = contextlib.ExitStack()
        with ctx:
            wp = ctx.enter_context(tc.tile_pool(name="wp", bufs=2))
            ap_ = ctx.enter_context(tc.tile_pool(name="ap", bufs=1))
            bcp = ctx.enter_context(tc.tile_pool(name="bcp", bufs=16))
            sp = ctx.enter_context(tc.tile_pool(name="sp", bufs=8))
            cp = ctx.enter_context(tc.tile_pool(name="cp", bufs=1))
            pbig = ctx.enter_context(tc.tile_pool(name="pbig", bufs=3, space="PSUM"))
            ph = ctx.enter_context(tc.tile_pool(name="ph", bufs=2, space="PSUM"))
            pt = ctx.enter_context(tc.tile_pool(name="pt", bufs=2, space="PSUM"))

            # ---------------- constants ----------------
            ident = cp.tile([128, 128], F16)
            nc.sync.dma_start(out=ident, in_=d["ident"].ap())
            epsc = cp.tile([128, 1], F32)
            nc.vector.memset(epsc, EPS)
            dum = sp.tile([128, 1], F32, tag="dum", bufs=4)
            # prewarm the exp table set before the first corr softmax
            nc.scalar.activation(dum, epsc, AT.Exp)
            memb_sb = cp.tile([128, KD, DM], F16)
            nc.sync.dma_start(out=memb_sb, in_=d["memb"].ap())
            wpos_sb = cp.tile([128, DM], F32)
            nc.sync.dma_start(out=wpos_sb, in_=d["wpos"].ap())
            xin_sb = cp.tile([128, BPC, KD, C], F16)
            nc.sync.dma_start(out=xin_sb, in_=d["xin"].ap())

            # ---------------- embed:  x[c] = xin[c].T @ memb + wpos ----------------
            x_t = ap_.tile([128, BPC, DM], F16, tag="xa", bufs=2)
            for c in range(BPC):
                x_ps = pbig.tile([128, DM], F32, tag="big")
                for k in range(KD):
                    mm(x_ps, xin_sb[:, c, k, :], memb_sb[:, k, :],
                       start=(k == 0), stop=(k == KD - 1))
                nc.vector.tensor_add(x_t[:, c, :], x_ps, wpos_sb)

            phase = os.environ.get("KPHASE", "full")
            srow_of = {}

            # ---------------- layers ----------------
            for l in range(L if phase == "full" else 1):
                if phase == "emb":
                    break
                # ---- layer weight loads (wp bufs=2 -> prefetch overlap) ----
                vw1t = wp.tile([128, KD, H], F16, tag="vw1t")
                nc.sync.dma_start(out=vw1t, in_=d["vw1t"][l])
                vb1 = sp.tile([128, KH], F32, tag="vb1", bufs=2)
                nc.sync.dma_start(out=vb1, in_=d["vb1"][l])
                vw2t = wp.tile([128, KH, DM], F16, tag="vw2t")
                nc.sync.dma_start(out=vw2t, in_=d["vw2t"][l])
                m1 = wp.tile([128, KD, DM], F16, tag="m1")
                nc.sync.dma_start(out=m1, in_=d["m1"][l])
                m2 = wp.tile([128, KD, DM], F16, tag="m2")
                nc.sync.dma_start(out=m2, in_=d["m2"][l])
                aw1t = wp.tile([128, KD, H], F16, tag="aw1t")
                nc.sync.dma_start(out=aw1t, in_=d["aw1t"][l])
                ab1 = sp.tile([128, KH], F32, tag="ab1", bufs=2)
                nc.sync.dma_start(out=ab1, in_=d["ab1"][l])
                aw2t = wp.tile([128, KH, DM], F16, tag="aw2t")
                nc.sync.dma_start(out=aw2t, in_=d["aw2t"][l])

                vgc = sp.tile([128, 1], F32, tag="vgc", bufs=2)
                nc.gpsimd.dma_start(out=vgc, in_=col_ap(d["vgc"][l]))
                agc = sp.tile([128, 1], F32, tag="agc", bufs=2)
                nc.gpsimd.dma_start(out=agc, in_=col_ap(d["agc"][l]))
                vbc = sp.tile([128, 1], F32, tag="vbc", bufs=2)
                nc.gpsimd.dma_start(out=vbc, in_=col_ap(d["vbc"][l]))
                abc = sp.tile([128, 1], F32, tag="abc", bufs=2)
                nc.gpsimd.dma_start(out=abc, in_=col_ap(d["abc"][l]))
                vbcf = bcp.tile([128, 128], F16, tag="bc2", name=f"vbcf{l}")
                nc.gpsimd.dma_start(out=vbcf, in_=bc_ap(d["vbch"][l]))

                def bcast(name):
                    t = bcp.tile([128, DM], F16, tag="bc", name=f"{name}_bc{l}")
                    nc.gpsimd.dma_start(out=t, in_=bc_ap(d[name][l]))
                    return t
                c1b = bcast("c1"); c2b = bcast("c2")
                vb2b = bcast("vb2"); ab2b = bcast("ab2")
                vglb = bcast("vgl"); vblb = bcast("vbl")
                aglb = bcast("agl"); ablb = bcast("abl")

                # gcI = diag(gc_vc) as dense tile for the "+I" residual fold
                gcI = sp.tile([128, 128], F16, tag="gcI", bufs=2)
                nc.vector.tensor_scalar_mul(gcI, ident, vgc)
                vsb = sp.tile([128, 1], F32, tag="vsb", bufs=2)
                nc.gpsimd.dma_start(out=vsb, in_=bc_ap(d["vsb"][l]))
                asb = sp.tile([128, 1], F32, tag="asb", bufs=2)
                nc.gpsimd.dma_start(out=asb, in_=bc_ap(d["asb"][l]))

                # ============ VarCor block ============
                # s = rowsum(x) * 1/DM^0.75 (split sqrt per side)
                cT = ap_.tile([128, BPC, 128], F16, tag="cT", bufs=2)
                for c in range(BPC):
                    if c in srow_of:
                        srow = srow_of[c]
                    else:
                        srow = sp.tile([128, 1], F32, tag="srow", bufs=4)
                        nc.vector.tensor_reduce(srow, x_t[:, c, :], AX.X, ALU.add)
                    s2 = sp.tile([128, 1], F16, tag="s2", bufs=4)
                    nc.vector.tensor_scalar_mul(s2, srow, inv_sqc)
                    sT_ps = pt.tile([1, 128], F16, tag="t", name=f"sTps{l}_{c}")
                    nc.tensor.transpose(sT_ps, s2, ident)
                    sT = sp.tile([1, 128], F16, tag="sT", bufs=4)
                    nc.scalar.activation(sT, sT_ps, AT.Identity)
                    corr_ps = pbig.tile([128, 128], F32, tag="big", name=f"corrps{l}_{c}")
                    mm(corr_ps, sT, sT, start=True, stop=True)
                    # softmax over free axis (values are O(1): skip max-sub)
                    # + BN row-scale + +I fold
                    corrE = ap_.tile([128, 128], F32, tag="corrE", bufs=2)
                    rsum = sp.tile([128, 1], F32, tag="rsum", bufs=4)
                    nc.scalar.activation(corrE, corr_ps, AT.Exp, accum_out=rsum)
                    rinv = sp.tile([128, 1], F32, tag="rinv", bufs=4)
                    nc.vector.reciprocal(rinv, rsum)
                    corrBN = ap_.tile([128, 128], F16, tag="corrBN", bufs=2)
                    nc.vector.tensor_scalar(corrBN, corrE, rinv, vgc, ALU.mult, ALU.mult)
                    nc.vector.tensor_add(corrBN, corrBN, gcI)
                    cT_ps = pt.tile([128, 128], F16, tag="t", name=f"cTps{l}_{c}")
                    nc.tensor.transpose(cT_ps, corrBN, ident)
                    nc.vector.tensor_copy(cT[:, c, :], cT_ps)

                # r2 rows-major and feature-major via two matmul sets
                r2r = ap_.tile([128, BPC, DM], F16, tag="r2r", bufs=2)
                r2T = ap_.tile([128, KD, 2 * 128], F16, tag="r2T", bufs=2)
                for c in range(BPC):
                    rr_ps = pbig.tile([128, DM], F32, tag="big", name=f"rrps{l}_{c}")
                    mm(rr_ps, cT[:, c, :], x_t[:, c, :DM], start=True, stop=True)
                    nc.scalar.activation(r2r[:, c, :], rr_ps, AT.Identity, bias=vbc)
                    for m in range(KD):
                        rt_ps = pt.tile([128, 128], F32, tag="t", name=f"rtps{l}_{c}_{m}")
                        mm(rt_ps, x_t[:, c, m * 128:(m + 1) * 128],
                           cT[:, c, :], start=True, stop=True)
                        # feature-major r2T: BN beta is along the free (channel)
                        # axis here, so add it via a partition-broadcast tile
                        # (GPSIMD cannot read PSUM, so these stay on DVE)
                        nc.vector.tensor_add(r2T[:, m, c * 128:(c + 1) * 128],
                                             rt_ps, vbcf)

                if phase == "corr":
                    x_t = r2r
                    break
                x_t = _ffn_ln(nc, tile, mybir, bass, tc, ap_, sp, bcp, ph, pbig,
                              r2T, r2r, vw1t, vb1, vw2t, vb2b, vglb, vblb, l, "v", epsc,
                              vsb, srow_of, dup=True, last=False)
                if phase == "vc0":
                    break

                # ============ Auto-attention block ============
                # x_t is [128, BPC, 2*DM] ([x, x] duplicated): window sh:sh+DM
                # is roll_sh(x). Per-batch software pipeline: while batch c's
                # scores grind through the DVE, the PE runs batch c-1's value
                # matmuls and batch c+1 has nothing pending, so the 9.7us
                # per-layer PE bubble of the batch-synchronous order closes.
                scl = float(DM ** -0.5)
                xT = ap_.tile([128, KD, 2 * 128], F16, tag="xT", bufs=2)
                u_t = ap_.tile([128, BPC, DM], F16, tag="u", bufs=2)
                Sp_t = ap_.tile([128, BPC, NS], F16, tag="Sp", bufs=2)
                vm_t = ap_.tile([128, BPC, DM], F16, tag="vm", bufs=2)

                def attn_head(c):
                    # xT feature-major + u = x @ M1 + c1 (rows-major)
                    for m in range(KD):
                        tp = pt.tile([128, 128], F16, tag="t", name=f"xTps{l}_{c}_{m}")
                        nc.tensor.transpose(tp, x_t[:, c, m * 128:(m + 1) * 128], ident)
                        if m % 2 == 0:
                            nc.vector.tensor_copy(xT[:, m, c * 128:(c + 1) * 128], tp)
                        else:
                            nc.scalar.activation(xT[:, m, c * 128:(c + 1) * 128], tp,
                                                 AT.Identity)
                    u_ps = pbig.tile([128, DM], F32, tag="big", name=f"ups{l}_{c}")
                    for k in range(KD):
                        mm(u_ps, xT[:, k, c * 128:(c + 1) * 128],
                           m1[:, k, :], start=(k == 0), stop=(k == KD - 1))
                    nc.vector.tensor_add(u_t[:, c, :], u_ps, c1b)

                def attn_scores(c):
                    # S[r,i] = <u, roll_i(x)> * DM^-0.5 ; softmax over i
                    # NOTE: tensor_tensor_reduce wedges the device on this
                    # walrus/NRT build (NRT_EXEC_UNIT_UNRECOVERABLE); use
                    # scalar_tensor_tensor's accum_out instead.
                    Sa = sp.tile([128, NS], F32, tag="Sa", bufs=2)
                    for i in range(NS):
                        trash = ap_.tile([128, DM], F16, tag="trash", bufs=2,
                                         name=f"tr{l}_{c}_{i}")
                        nc.vector.scalar_tensor_tensor(
                            out=trash, in0=u_t[:, c, :], scalar=scl,
                            in1=x_t[:, c, P * i:P * i + DM],
                            op0=ALU.mult, op1=ALU.mult, accum_out=Sa[:, i:i + 1])
                    Se = sp.tile([128, NS], F32, tag="Se", bufs=2)
                    ssum = sp.tile([128, 1], F32, tag="ssum", bufs=4)
                    nc.scalar.activation(Se, Sa, AT.Exp, accum_out=ssum)
                    sinv = sp.tile([128, 1], F32, tag="sinv", bufs=4)
                    nc.vector.reciprocal(sinv, ssum)
                    nc.vector.tensor_scalar(Sp_t[:, c, :], Se, sinv, None, ALU.mult)

                def attn_values(c):
                    # vm = sum_i p_i roll_i(x) via diag matmuls in PSUM.
                    # All 8 diag(p_i) tiles are built in ONE tensor_tensor:
                    # ident repeated 8x (0-stride dim) times Sp broadcast
                    # along the 128-column dim.
                    dg_all = ap_.tile([128, NS * 128], F16, tag="dg", bufs=2,
                                      name=f"dg{l}_{c}")
                    id_rep = bass.AP(tensor=ident.tensor, offset=ident.offset,
                                     ap=[list(ident.ap[0]), [0, NS], [1, 128]])
                    spc = Sp_t[:, c, :]
                    sp_rep = bass.AP(tensor=spc.tensor, offset=spc.offset,
                                     ap=[list(spc.ap[0]), [1, NS], [0, 128]])
                    dg3 = bass.AP(tensor=dg_all.tensor, offset=dg_all.offset,
                                  ap=[list(dg_all.ap[0]), [128, NS], [1, 128]])
                    nc.vector.tensor_tensor(out=dg3, in0=id_rep, in1=sp_rep,
                                            op=ALU.mult)
                    vm_ps = pbig.tile([128, DM], F32, tag="big", name=f"vmps{l}_{c}")
                    for i in range(NS):
                        mm(vm_ps, dg_all[:, i * 128:(i + 1) * 128],
                           x_t[:, c, P * i:P * i + DM],
                           start=(i == 0), stop=(i == NS - 1))
                    nc.vector.tensor_copy(vm_t[:, c, :], vm_ps)

                for c in range(BPC):
                    attn_head(c)
                attn_scores(0)
                attn_values(0)      # PE busy on batch 0 while DVE...
                attn_scores(1)      # ...grinds batch 1's reductions
                attn_values(1)

                if phase == "u":
                    x_t = u_t
                    break
                if phase == "sc":
                    xs = ap_.tile([128, BPC, DM], F32, tag="scdump", bufs=1)
                    nc.vector.memset(xs, 0.0)
                    for c in range(BPC):
                        nc.vector.tensor_copy(xs[:, c, 0:NS], Sp_t[:, c, :])
                    x_t = xs
                    break
                if phase == "vm":
                    x_t = vm_t
                    break

                # vmT feature-major
                vmT = ap_.tile([128, KD, 2 * 128], F16, tag="vmT", bufs=2)
                for c in range(BPC):
                    for m in range(KD):
                        tp2 = pt.tile([128, 128], F16, tag="t", name=f"vmTps{l}_{c}_{m}")
                        nc.tensor.transpose(tp2, vm_t[:, c, m * 128:(m + 1) * 128], ident)
                        if (c * KD + m) % 2 == 0:
                            nc.vector.tensor_copy(vmT[:, m, c * 128:(c + 1) * 128], tp2)
                        else:
                            nc.scalar.activation(vmT[:, m, c * 128:(c + 1) * 128], tp2,
                                                 AT.Identity)

                # attn out rows-major: o = vm @ M2 + c2 ; r1 = BN(o + x)
                r1r = ap_.tile([128, BPC, DM], F16, tag="r1r", bufs=2)
                for c in range(BPC):
                    o_ps = pbig.tile([128, DM], F32, tag="big", name=f"ops{l}_{c}")
                    for k in range(KD):
                        mm(o_ps, vmT[:, k, c * 128:(c + 1) * 128],
                           m2[:, k, :], start=(k == 0), stop=(k == KD - 1))
                    t1 = ap_.tile([128, DM], F32, tag="t1", bufs=2, name=f"t1{l}_{c}")
                    nc.vector.tensor_add(t1, o_ps, x_t[:, c, :DM])
                    nc.vector.tensor_add(t1, t1, c2b)
                    nc.scalar.activation(r1r[:, c, :], t1, AT.Identity, bias=abc, scale=agc)

                if phase == "attn":
                    x_t = r1r
                    break

                # r1T feature-major
                r1T = ap_.tile([128, KD, 2 * 128], F16, tag="r1T", bufs=2)
                for c in range(BPC):
                    for m in range(KD):
                        tp3 = pt.tile([128, 128], F16, tag="t", name=f"r1Tps{l}_{c}_{m}")
                        nc.tensor.transpose(tp3, r1r[:, c, m * 128:(m + 1) * 128], ident)
                        if (c * KD + m) % 2 == 0:
                            nc.vector.tensor_copy(r1T[:, m, c * 128:(c + 1) * 128], tp3)
                        else:
                            nc.scalar.activation(r1T[:, m, c * 128:(c + 1) * 128], tp3,
                                                 AT.Identity)

                x_t = _ffn_ln(nc, tile, mybir, bass, tc, ap_, sp, bcp, ph, pbig,
                              r1T, r1r, aw1t, ab1, aw2t, ab2b, aglb, ablb, l, "a", epsc,
                              asb, srow_of, dup=False, last=(l == L - 1))

            # ---------------- store ----------------
            if x_t.dtype != F32:
                xf = ap_.tile([128, BPC, DM], F32, tag="xf32", bufs=1)
                for c in range(BPC):
                    nc.vector.tensor_copy(xf[:, c, :], x_t[:, c, :DM])
                x_t = xf
            for c in range(BPC):
                nc.sync.dma_start(out=out_d.ap()[c], in_=x_t[:, c, :DM])


def _ffn_ln(nc, tile, mybir, bass, tc, ap_, sp, bcp, ph, pbig,
            rT, rrows, w1t, b1, w2t, b2b, glb, blb, l, pfx, epsc,
            sumb, srow_of, dup, last):
    """h = gelu(r @ W1.T + b1); y = h @ W2.T + b2; x = LN(y + r) * g + b.

    LN stats: the z-producing stt accumulates sum(z); an ACT Square pass
    accumulates sum(z^2); var = E[z^2] - mu^2; rstd = exp(-0.5*ln(var+eps))
    (ln/exp/square/identity live in one ACT table set).

    dup=True: write x twice side by side ([x, x], free 2*DM) so circular
    rolls of the following attention block are contiguous windows.

    Also emits (for the "a" blocks feeding the next varcor) the row-sum of
    the next x via <xn, g> + sum(b) so the correlation chain never waits on
    the gamma/beta affine."""
    F32 = mybir.dt.float32
    F16 = mybir.dt.float16
    AT = mybir.ActivationFunctionType
    ALU = mybir.AluOpType

    # rb = r + b2 precomputed off the critical path while FFN runs
    rb = ap_.tile([128, BPC, DM], F16, tag=f"rb{pfx}", bufs=2, name=f"rb{pfx}{l}")
    for c in range(BPC):
        nc.vector.tensor_add(rb[:, c, :], rrows[:, c, :], b2b)

    # prewarm the Gelu table set while the first FFN1 matmuls run
    dg_ = sp.tile([128, 1], F32, tag="dum", bufs=4, name=f"dumg{pfx}{l}")
    nc.scalar.activation(dg_, epsc, AT.Gelu)

    hT = ap_.tile([128, KH, 2 * 128], F16, tag="hT", bufs=2, name=f"hT{pfx}{l}")
    for mh2 in range(KH // 2):
        h_ps = ph.tile([128, 2, 128 * 2], F32, tag="h", name=f"hps{pfx}{l}_{mh2}")
        for half in range(2):
            mh = mh2 * 2 + half
            for k in range(KD):
                nc.tensor.matmul(h_ps[:, half, :], w1t[:, k, mh * 128:(mh + 1) * 128],
                                 rT[:, k, :], start=(k == 0), stop=(k == KD - 1))
            nc.scalar.activation(hT[:, mh, :], h_ps[:, half, :], AT.Gelu,
                                 bias=b1[:, mh:mh + 1])
    # prewarm the sqrt set back in while FFN2 matmuls run (square is in
    # every table set, so the Square stats pass below never switches)
    dn_ = sp.tile([128, 1], F32, tag="dum", bufs=4, name=f"dumn{pfx}{l}")
    nc.scalar.activation(dn_, epsc, AT.Sqrt)

    out_w = 2 * DM if dup else DM
    out_dt = F32 if last else F16
    x_new = ap_.tile([128, BPC, out_w], out_dt, tag=f"x{pfx}{'d' if dup else ''}",
                     bufs=2, name=f"x{pfx}{l}")
    for c in range(BPC):
        y_ps = pbig.tile([128, DM], F32, tag="big", name=f"yps{pfx}{l}_{c}")
        for k in range(KH):
            nc.tensor.matmul(y_ps, hT[:, k, c * 128:(c + 1) * 128],
                             w2t[:, k, :], start=(k == 0), stop=(k == KH - 1))
        # z = y + r + b2 (one stt, accumulating sum(z) for the LN mean)
        z = ap_.tile([128, DM], F16, tag="z", bufs=4, name=f"z{pfx}{l}_{c}")
        zsum = sp.tile([128, 1], F32, tag="zsum", bufs=4)
        nc.vector.scalar_tensor_tensor(
            out=z, in0=y_ps, scalar=1.0, in1=rb[:, c, :],
            op0=ALU.mult, op1=ALU.add, accum_out=zsum)
        # sum(z^2) on the ACT engine (square is in every table set)
        ztr = ap_.tile([128, DM], F16, tag="ztr", bufs=2, name=f"ztr{pfx}{l}_{c}")
        z2sum = sp.tile([128, 1], F32, tag="z2sum", bufs=4)
        nc.scalar.activation(ztr, z, AT.Square, accum_out=z2sum)
        mu = sp.tile([128, 1], F32, tag="mu", bufs=4)
        nc.vector.tensor_scalar_mul(mu, zsum, float(1.0 / DM))
        nb = sp.tile([128, 1], F32, tag="nb", bufs=4)
        nc.vector.tensor_scalar_mul(nb, mu, -1.0)
        mu2 = sp.tile([128, 1], F32, tag="mu2", bufs=4)
        nc.vector.tensor_mul(mu2, mu, mu)
        var = sp.tile([128, 1], F32, tag="var", bufs=4)
        nc.vector.scalar_tensor_tensor(
            out=var, in0=z2sum, scalar=float(1.0 / DM), in1=mu2,
            op0=ALU.mult, op1=ALU.subtract)
        # rstd = 1/sqrt(var + eps)  (sqrt prewarmed above; recip is native DVE)
        std = sp.tile([128, 1], F32, tag="std", bufs=4)
        nc.scalar.activation(std, var, AT.Sqrt, bias=epsc)
        rstd = sp.tile([128, 1], F32, tag="rstd", bufs=4)
        nc.vector.reciprocal(rstd, std)
        xn = ap_.tile([128, DM], F16, tag="xn", bufs=2, name=f"xn{pfx}{l}_{c}")
        nc.vector.tensor_scalar(xn, z, nb, rstd, ALU.add, ALU.mult)
        if pfx == "a" and l < L - 1:
            # next-layer corr row-sum: <xn, g> + sum(b) — skips the affine
            trash2 = ap_.tile([128, DM], F16, tag="tr2", bufs=2,
                              name=f"tr2{pfx}{l}_{c}")
            sraw = sp.tile([128, 1], F32, tag="sraw", bufs=4)
            nc.vector.scalar_tensor_tensor(
                out=trash2, in0=xn, scalar=1.0, in1=glb,
                op0=ALU.mult, op1=ALU.mult, accum_out=sraw)
            srow = sp.tile([128, 1], F32, tag="srow", bufs=4, name=f"srow{pfx}{l}_{c}")
            nc.scalar.activation(srow, sraw, AT.Identity, bias=sumb)
            srow_of[c] = srow
        # affine (the next-layer corr chain does not wait on it: srow above)
        nc.vector.tensor_mul(x_new[:, c, :DM], xn, glb)
        nc.vector.tensor_add(x_new[:, c, :DM], x_new[:, c, :DM], blb)
        if dup:
            # second copy for contiguous roll windows (off critical path)
            nc.vector.tensor_copy(x_new[:, c, DM:], x_new[:, c, :DM])
    # prewarm the exp set for the following softmax (scores / next corr)
    de_ = sp.tile([128, 1], F32, tag="dum", bufs=4, name=f"dume{pfx}{l}")
    nc.scalar.activation(de_, epsc, AT.Exp)
    return x_new


# ======================================================================
# host side
# ======================================================================

_COMPILED = {}


def _compile():
    if "nc" in _COMPILED:
        return _COMPILED["nc"]
    import concourse.bass as bass
    import concourse.bacc as bacc
    import concourse.tile as tile
    from concourse import mybir
    nc = bacc.Bacc("TRN2", target_bir_lowering=False, debug=False, num_devices=NC_)
    _build(nc, tile, mybir, bass)
    nc.compile()
    _COMPILED["nc"] = nc
    return nc


def _host_prep(inputs):
    f = lambda k: np.asarray(inputs[k], np.float32)
    ld_w = f("ld_w").reshape(KS).astype(np.float64)
    # conv matrix with replicate padding, R = I - S
    S = np.zeros((T, T), np.float64)
    idx = np.clip(np.arange(T)[:, None] + np.arange(KS)[None, :] - KS // 2, 0, T - 1)
    for k in range(KS):
        np.add.at(S, (np.arange(T), idx[:, k]), ld_w[k])
    Rm = np.eye(T) - S
    emb_W = f("emb_W").astype(np.float64)
    memb = (Rm.T @ emb_W.T).astype(np.float16)              # (T, DM)
    wpos = (f("W_pos") + f("emb_b")[None, :]
            - float(f("ld_b")[0]) * emb_W.sum(1).astype(np.float32)[None, :])

    g = {"memb": np.ascontiguousarray(memb.reshape(KD, 128, DM).transpose(1, 0, 2)),
         "wpos": np.ascontiguousarray(wpos.astype(np.float32)),
         "ident": np.eye(128, dtype=np.float16)}

    s1 = np.float32(1.0 / np.sqrt(1.0 + EPS))
    def stack(fn, dt=np.float32):
        return np.ascontiguousarray(np.stack([fn(l) for l in range(L)]).astype(dt))

    def shuf(a):
        # (k*128, n) -> (128, k, n): SBUF layout with contiguous per-partition rows
        kn, n = a.shape
        return a.reshape(kn // 128, 128, n).transpose(1, 0, 2)

    h16 = np.float16
    g["vw1t"] = stack(lambda l: shuf(f("vc_W1")[l].T), h16)
    g["vb1"] = stack(lambda l: f("vc_b1")[l].reshape(KH, 128).T)
    g["vw2t"] = stack(lambda l: shuf(f("vc_W2")[l].T), h16)
    g["vb2"] = stack(lambda l: f("vc_b2")[l], h16)
    g["aw1t"] = stack(lambda l: shuf(f("aa_W1")[l].T), h16)
    g["ab1"] = stack(lambda l: f("aa_b1")[l].reshape(KH, 128).T)
    g["aw2t"] = stack(lambda l: shuf(f("aa_W2")[l].T), h16)
    g["ab2"] = stack(lambda l: f("aa_b2")[l], h16)
    g["m1"] = stack(lambda l: shuf(f("aa_Wq")[l].astype(np.float64).T @ f("aa_Wk")[l].astype(np.float64)), h16)
    g["c1"] = stack(lambda l: f("aa_bq")[l].astype(np.float64) @ f("aa_Wk")[l].astype(np.float64), h16)
    g["m2"] = stack(lambda l: shuf((f("aa_Wo")[l].astype(np.float64) @ f("aa_Wv")[l].astype(np.float64)).T), h16)
    g["c2"] = stack(lambda l: f("aa_bv")[l].astype(np.float64) @ f("aa_Wo")[l].astype(np.float64).T
                    + f("aa_bo")[l].astype(np.float64), h16)
    g["vsb"] = stack(lambda l: f("vc_ln_b")[l].sum(keepdims=True))
    g["asb"] = stack(lambda l: f("aa_ln_b")[l].sum(keepdims=True))
    g["vgc"] = stack(lambda l: f("vc_bn_g")[l] * s1)
    g["vbc"] = stack(lambda l: f("vc_bn_b")[l])
    g["vbch"] = stack(lambda l: f("vc_bn_b")[l], h16)
    g["vgl"] = stack(lambda l: f("vc_ln_g")[l], h16)
    g["vbl"] = stack(lambda l: f("vc_ln_b")[l], h16)
    g["agc"] = stack(lambda l: f("aa_bn_g")[l] * s1)
    g["abc"] = stack(lambda l: f("aa_bn_b")[l])
    g["agl"] = stack(lambda l: f("aa_ln_g")[l], h16)
    g["abl"] = stack(lambda l: f("aa_ln_b")[l], h16)
    return g


def kernel(**inputs):
    from concourse.bass_utils import run_bass_kernel_spmd
    nc = _compile()
    g = _host_prep(inputs)
    inp = np.asarray(inputs["inp"], np.float32)
    in_maps = []
    for core in range(NC_):
        m = dict(g)
        sl = inp[core * BPC:(core + 1) * BPC]          # (BPC, T, C)
        m["xin"] = np.ascontiguousarray(
            sl.reshape(BPC, KD, 128, C).transpose(2, 0, 1, 3)).astype(np.float16)
        in_maps.append(m)
    res = run_bass_kernel_spmd(nc, in_maps, core_ids=list(range(NC_)))
    if res.exec_time_ns is not None:
        kernel.last_exec_time_ns = res.exec_time_ns
    out = np.concatenate([res.results[k]["out"] for k in range(NC_)], axis=0)
    return out


kernel.last_exec_time_ns = None
